# revision 26
# baseline (speedup 1.0000x reference)
"""Trainium2 Bass kernel: 4-head causal+ragged attention, one sample per core.

bf16 datapath: QKV projection, scores (QK^T with causal fixup via tril matmul),
exp on ScalarE with per-key-block mask bias, AV accumulation with a fused
ones-row denominator, reciprocal on a gathered [4, FILL] tile, and output
projection. PSUM accumulation stays fp32. Host wrapper verifies the mask is
causal & key-length structured, shards one sample per core, and gathers.
Falls back to a pure-numpy reference path for unstructured masks.
"""
import sys
sys.path.insert(0, '/opt/trn_rl_repo')
import numpy as np
import ml_dtypes
import concourse.bacc as bacc
import concourse.mybir as mybir
from concourse.tile import TileContext
from concourse.dve_ops import RECIP_APPROX_FAST_CONSTS, RECIPROCAL_APPROX_FAST

F32 = mybir.dt.float32
F32R = mybir.dt.float32r
BF16 = mybir.dt.bfloat16
EXP = mybir.ActivationFunctionType.Exp

S = 2048
D = 64
H = 4
DH = 16
NKB = S // 128
FILL = 1024
NEG = -1e30


def build_nc(num_cores=8, loop_n=1, maxlen=S, dbg=False):
    kb_max = (int(maxlen) + 127) // 128 - 1  # last key block any sample attends
    nc = bacc.Bacc("TRN2", target_bir_lowering=False, debug=False, num_devices=num_cores)
    QT = nc.dram_tensor("qt", [128, S], BF16, kind="ExternalInput").ap()
    KT = nc.dram_tensor("kt", [128, S], BF16, kind="ExternalInput").ap()
    QT3 = nc.dram_tensor("qt3", [DH, S], BF16, kind="ExternalInput").ap()
    KT3 = nc.dram_tensor("kt3", [DH, S], BF16, kind="ExternalInput").ap()
    V4 = nc.dram_tensor("v4", [128, 2 * S], BF16, kind="ExternalInput").ap()
    MTRIL = nc.dram_tensor("mtril", [128, 128], BF16, kind="ExternalInput").ap()
    KBIAS = nc.dram_tensor("kbias", [128, NKB], F32, kind="ExternalInput").ap()
    WP = nc.dram_tensor("wp", [128, D], BF16, kind="ExternalInput").ap()
    EB4 = nc.dram_tensor("eb4", [H, 128], F32R, kind="ExternalInput").ap()
    Y = nc.dram_tensor("y", [S, D], F32, kind="ExternalOutput").ap()
    if dbg:
        DQK = nc.dram_tensor("d_qk", [128, 2 * S], BF16, kind="ExternalOutput").ap()
        DV = nc.dram_tensor("d_v", [128, 2 * S], BF16, kind="ExternalOutput").ap()
        DCTX = nc.dram_tensor("d_ctx", [128, S], BF16, kind="ExternalOutput").ap()
        DR4 = nc.dram_tensor("d_r4", [H, S], F32, kind="ExternalOutput").ap()
        DZ = nc.dram_tensor("d_z", [128, S], BF16, kind="ExternalOutput").ap()
        DEX = nc.dram_tensor("d_ex", [128, S], BF16, kind="ExternalOutput").ap()

    import contextlib
    with TileContext(nc) as tc, nc.allow_low_precision(
            reason="bf16 datapath intended; accumulation stays fp32 in PSUM"):
        loop_cm = tc.For_i(0, loop_n, 1) if loop_n > 1 else contextlib.nullcontext()
        with loop_cm, \
             tc.sbuf_pool(name="const", bufs=1) as cp, \
             tc.sbuf_pool(name="sb", bufs=1) as sp, \
             tc.sbuf_pool(name="ex", bufs=14) as ep:
            mtril = cp.tile([128, 128], BF16)
            nc.sync.dma_start(out=mtril[:], in_=MTRIL[:])
            kbias = cp.tile([128, NKB], F32)
            nc.sync.dma_start(out=kbias[:], in_=KBIAS[:])
            wp = cp.tile([128, D], BF16)
            nc.sync.dma_start(out=wp[:], in_=WP[:])
            eb4 = cp.tile([H, 128], F32R)
            nc.sync.dma_start(out=eb4[:], in_=EB4[:])

            # host-precomputed projections: heads packed at 32-row stripes;
            # head 3 in base-0 tiles (PE operand base partition must be 0/32/64)
            qT_s = sp.tile([128, S], BF16)
            nc.sync.dma_start(out=qT_s[:], in_=QT[:])
            kT_s = sp.tile([128, S], BF16)
            nc.sync.dma_start(out=kT_s[:], in_=KT[:])
            qT_s3 = sp.tile([DH, S], BF16)
            nc.sync.dma_start(out=qT_s3[:], in_=QT3[:])
            kT_s3 = sp.tile([DH, S], BF16)
            nc.sync.dma_start(out=kT_s3[:], in_=KT3[:])
            # per kb block: 4 head-blocks of 64 cols; head h nonzero only in
            # its 32-col stripe so pair-overlapped AV accumulation adds zeros
            v_s = sp.tile([128, 2 * S], BF16)
            nc.sync.dma_start(out=v_s[:], in_=V4[:, 0:2 * S])

            if dbg:
                nc.sync.dma_start(out=DQK[:, 0:S], in_=qT_s[:])
                nc.sync.dma_start(out=DQK[:, S:2 * S], in_=kT_s[:])
                nc.sync.dma_start(out=DV[:], in_=v_s[:])
            # packed denominators (from pctx rows 0,1,64,65 via ctx_s + DMA
            # partition shift -- the custom recip DVE op misbehaves at
            # partition offsets > 0, so everything runs at offset 0)
            den2 = sp.tile([H, S], BF16)
            den4 = sp.tile([H, S], F32)
            R4g = sp.tile([H, S], F32R)

            with tc.psum_pool(name="ps", bufs=3) as ps, \
                 tc.psum_pool(name="pc", bufs=1) as pc:
                fstate = {}

                # finalize is split in three stages injected into the NEXT
                # q-chunk's unit stream: (a) drain pctx to SBUF + reciprocal
                # right away (frees the single pctx buffer for reuse), (b) the
                # recip broadcast matmul once the PE has other work queued,
                # (c) the output projection once z is surely ready.
                def recip_f32r(out, in_):
                    c = RECIP_APPROX_FAST_CONSTS
                    nc.vector._custom_dve(RECIPROCAL_APPROX_FAST, out=out,
                                          in0=in_, s0=c["s0"], s1=c["s1"],
                                          imm2=c["imm2"])

                def emit_finalize_a(qc, pctx):
                    qlo = qc * FILL
                    ctx_s = sp.tile([128, FILL], BF16, tag="ctxs", bufs=2)
                    nc.vector.tensor_copy(ctx_s[:], pctx[:])
                    nc.sync.dma_start(out=den2[0:2, qlo:qlo + FILL],
                                      in_=ctx_s[0:2, :])
                    nc.sync.dma_start(out=den2[2:4, qlo:qlo + FILL],
                                      in_=ctx_s[64:66, :])
                    nc.vector.tensor_copy(den4[:, qlo:qlo + FILL],
                                          den2[:, qlo:qlo + FILL])
                    recip_f32r(R4g[:, qlo:qlo + FILL],
                               den4[:, qlo:qlo + FILL])
                    if dbg:
                        nc.sync.dma_start(out=DCTX[:, qlo:qlo + FILL], in_=ctx_s[:])
                        nc.sync.dma_start(out=DR4[:, qlo:qlo + FILL],
                                          in_=R4g[:, qlo:qlo + FILL].bitcast(F32))

                    fstate[qc] = ctx_s

                def emit_finalize_b(qc):
                    qlo = qc * FILL
                    ctx_s = fstate[qc]
                    rb = ps.tile([128, FILL], F32, tag="st")
                    for c in range(FILL // 512):
                        lo = c * 512
                        nc.tensor.matmul(rb[:, lo:lo + 512], eb4[:],
                                         R4g[:, qlo + lo:qlo + lo + 512],
                                         start=True, stop=True)
                    rbs = sp.tile([128, FILL], BF16, tag="rbs", bufs=2)
                    nc.vector.tensor_copy(rbs[:], rb[:])
                    z = sp.tile([128, FILL], BF16, tag="z", bufs=2)
                    nc.vector.tensor_mul(z[:], ctx_s[:], rbs[:])
                    if dbg:
                        nc.sync.dma_start(out=DZ[:, qlo:qlo + FILL], in_=z[:])
                    fstate[qc] = z

                def emit_finalize_c(qc):
                    qlo = qc * FILL
                    z = fstate.pop(qc)
                    for t in range(FILL // 128):
                        t0 = qlo + t * 128
                        py = ps.tile([128, D], F32, tag="st")
                        nc.tensor.matmul(py[:], z[:, t * 128:(t + 1) * 128], wp[:],
                                         start=True, stop=True)
                        ys = sp.tile([128, D], F32, tag="ys", bufs=3)
                        nc.vector.tensor_copy(ys[:], py[:])
                        nc.sync.dma_start(out=Y[t0:t0 + 128, :], in_=ys[:])

                nqc = S // FILL
                prev_pctx = None
                for qc in range(nqc):
                    qlo, qhi = qc * FILL, (qc + 1) * FILL
                    if prev_pctx is not None:
                        emit_finalize_a(qc - 1, prev_pctx)
                    pctx = pc.tile([128, FILL], F32, tag="pctx")
                    prev_pctx = pctx
                    kbs = [kb for kb in range(min(qhi // 128, kb_max + 1))]
                    pendings = []  # delayed AV batches: (exs, n0, kb, last)

                    def emit_av_batch(pending):
                        pexs, pn0, pkb, plast = pending
                        q0p = 128 * pkb
                        for ph in (0, 2, 1, 3):  # alternate col groups: LDW overlap
                            p = ph // 2
                            for c in range(FILL // 512):
                                s0 = qlo + c * 512
                                if s0 + 512 <= q0p:
                                    continue
                                b0 = max(s0, q0p) - qlo
                                nc.tensor.matmul(
                                    pctx[64 * p:64 * p + 64, b0:(c + 1) * 512],
                                    v_s[:, 256 * pkb + 64 * ph:256 * pkb + 64 * ph + 64],
                                    pexs[ph][:, b0:(c + 1) * 512],
                                    start=(pkb == 0 and ph % 2 == 0),
                                    stop=(plast and ph % 2 == 1),
                                    skip_group_check=True)

                    for ki, kb in enumerate(kbs):
                        q0 = 128 * kb
                        n0 = max(q0 - qlo, 0)
                        diag = q0 >= qlo
                        exs = []
                        for h in range(H):  # scores back-to-back: one PE mode
                            if h < 3:
                                kT = kT_s[32 * h:32 * h + DH, q0:q0 + 128]
                                qT_f, h_lo = qT_s, h
                            else:
                                kT = kT_s3[:, q0:q0 + 128]
                                qT_f, h_lo = qT_s3, 0
                            st = ps.tile([128, FILL], F32, tag="st")
                            for c in range(FILL // 512):
                                s0 = qlo + c * 512
                                if s0 + 512 <= q0:
                                    continue
                                b0 = max(s0, q0) - qlo
                                nc.tensor.matmul(st[:, b0:(c + 1) * 512], kT,
                                                 qT_f[32 * h_lo:32 * h_lo + DH,
                                                      qlo + b0:s0 + 512],
                                                 start=True, stop=True)
                            ex = ep.tile([128, FILL], BF16, tag="ex")
                            nc.scalar.activation(ex[:, n0:FILL], st[:, n0:FILL],
                                                 EXP, bias=kbias[:, kb:kb + 1],
                                                 scale=0.25)
                            if diag:
                                # causal mask inside the diagonal 128-block:
                                # multiply by 0/1 mask on DVE (cheaper than a
                                # PE matmul that would thrash the tile mode)
                                nc.vector.tensor_mul(ex[:, n0:n0 + 128],
                                                     ex[:, n0:n0 + 128],
                                                     mtril[:])
                            exs.append(ex)
                        pendings.append((exs, n0, kb, kb == kbs[-1]))
                        if len(pendings) > 2:
                            emit_av_batch(pendings.pop(0))
                        # inject previous q-chunk's finalize once this chunk's
                        # pipeline is warm, so PE never idles on the recip chain
                        if qc > 0 and ki == 2:
                            emit_finalize_b(qc - 1)
                        if qc > 0 and ki == 3:
                            emit_finalize_c(qc - 1)
                    for pnd in pendings:
                        emit_av_batch(pnd)
                    pendings = []
                emit_finalize_a(nqc - 1, prev_pctx)
                emit_finalize_b(nqc - 1)
                emit_finalize_c(nqc - 1)
    nc.compile()
    return nc


def host_prep(x_b, lens_b, W_qkv, W_proj, b_proj):
    bf = ml_dtypes.bfloat16
    x_b = np.asarray(x_b, np.float32)
    q = x_b @ W_qkv[0:D].T                      # [S, D]
    k = x_b @ W_qkv[D:2 * D].T
    v = x_b @ W_qkv[2 * D:3 * D].T
    qT = np.zeros((128, S), bf)
    kT = np.zeros((128, S), bf)
    for h in range(H):
        qT[32 * h:32 * h + DH] = q[:, DH * h:DH * h + DH].T.astype(bf)
        kT[32 * h:32 * h + DH] = k[:, DH * h:DH * h + DH].T.astype(bf)
    qT3 = np.ascontiguousarray(qT[96:96 + DH])
    kT3 = np.ascontiguousarray(kT[96:96 + DH])
    # v4: per kb block of 256 cols, head h at 64h..64h+64; within the block:
    # even head: ones col at j=0, dims at j=2..18; odd head: ones at j=1,
    # dims at j=34..50; h0 also carries the bias row at j=19 (ones)
    v4 = np.zeros((128, 2 * S), bf)
    nkb = S // 128
    for kb in range(nkb):
        vb = v[kb * 128:(kb + 1) * 128]         # [128 keys, 64]
        for h in range(H):
            base = 256 * kb + 64 * h
            v4[:, base + h % 2] = 1.0
            jdim = 2 + 32 * (h % 2)
            v4[:, base + jdim:base + jdim + DH] = \
                vb[:, DH * h:DH * h + DH].astype(bf)
        v4[:, 256 * kb + 19] = 1.0
    j = np.arange(128)
    mtril = (j[:, None] <= j[None, :]).astype(bf)  # [key, q]: 1 = attend
    pos = np.arange(S)
    kbias = np.ascontiguousarray(
        np.where((pos < lens_b).reshape(NKB, 128).T, np.float32(0.0),
                 np.float32(NEG)))
    wp = np.zeros((128, D), bf)
    for h in range(H):
        r0 = 64 * (h // 2) + 2 + 32 * (h % 2)
        wp[r0:r0 + DH, :] = W_proj[:, DH * h:DH * h + DH].T.astype(bf)
    wp[19, :] = np.asarray(b_proj, np.float32).astype(bf)
    eb4 = np.zeros((H, 128), np.float32)
    for h in range(H):
        r0 = 64 * (h // 2)
        eb4[h, r0 + h % 2] = 1.0
        d0 = r0 + 2 + 32 * (h % 2)
        eb4[h, d0:d0 + DH] = 1.0
    eb4[0, 19] = 1.0
    return {"qt": qT, "kt": kT, "qt3": qT3, "kt3": kT3, "v4": v4,
            "mtril": mtril, "kbias": kbias, "wp": wp, "eb4": eb4}


_RUNNERS = {}


def _build_runner(nc, n_cores=8):
    import jax
    from jax.sharding import Mesh, PartitionSpec
    from jax.experimental.shard_map import shard_map
    from concourse.bass2jax import (_bass_exec_p, install_neuronx_cc_hook,
                                    partition_id_tensor)
    install_neuronx_cc_hook()
    partition_name = nc.partition_id_tensor.name if nc.partition_id_tensor else None
    in_names, out_names, out_avals, zero_outs = [], [], [], []
    for alloc in nc.m.functions[0].allocations:
        if not isinstance(alloc, mybir.MemoryLocationSet):
            continue
        name = alloc.memorylocations[0].name
        if alloc.kind == "ExternalInput":
            if name != partition_name:
                in_names.append(name)
        elif alloc.kind == "ExternalOutput":
            shape = tuple(alloc.tensor_shape)
            dtype = mybir.dt.np(alloc.dtype)
            out_names.append(name)
            out_avals.append(jax.core.ShapedArray(shape, dtype))
            zero_outs.append(np.zeros(shape, dtype))
    n_params = len(in_names)
    n_outs = len(out_avals)
    all_in_names = list(in_names) + list(out_names)
    if partition_name is not None:
        all_in_names.append(partition_name)
    donate = tuple(range(n_params, n_params + n_outs))

    def _body(*args):
        operands = list(args)
        if partition_name is not None:
            operands.append(partition_id_tensor())
        outs = _bass_exec_p.bind(
            *operands,
            out_avals=tuple(out_avals),
            in_names=tuple(all_in_names),
            out_names=tuple(out_names),
            lowering_input_output_aliases=(),
            sim_require_finite=True,
            sim_require_nnan=True,
            nc=nc,
        )
        return tuple(outs)

    devices = jax.devices()[:n_cores]
    mesh = Mesh(np.asarray(devices), ("core",))
    in_specs = (PartitionSpec("core"),) * (n_params + n_outs)
    out_specs = (PartitionSpec("core"),) * n_outs
    sharded = jax.jit(
        shard_map(_body, mesh=mesh, in_specs=in_specs, out_specs=out_specs,
                  check_rep=False),
        donate_argnums=donate, keep_unused=True)

    def run(in_maps):
        import jax
        per_core = [[np.asarray(m[n]) for n in in_names] for m in in_maps]
        concat_in = [np.concatenate([per_core[c][i] for c in range(n_cores)], axis=0)
                     for i in range(n_params)]
        concat_zeros = [np.zeros((n_cores * z.shape[0], *z.shape[1:]), z.dtype)
                        for z in zero_outs]
        out_arrs = sharded(*concat_in, *concat_zeros)
        jax.block_until_ready(out_arrs)
        return [
            {name: np.asarray(out_arrs[i]).reshape(n_cores, *out_avals[i].shape)[c]
             for i, name in enumerate(out_names)}
            for c in range(n_cores)
        ]
    return run


def _numpy_fallback(x, attn_mask, W_qkv, W_proj, b_proj):
    B, S_, D_ = x.shape
    qkv = x @ W_qkv.T
    qkv = qkv.reshape(B, S_, 3, H, DH).transpose(2, 0, 3, 1, 4)
    q, k, v = qkv[0], qkv[1], qkv[2]
    s = np.einsum('bhqd,bhkd->bhqk', q, k).astype(np.float32) / np.sqrt(DH)
    neg = np.finfo(np.float32).min
    s = np.where(attn_mask, s, neg)
    s = s - s.max(-1, keepdims=True)
    p = np.exp(s)
    p = p / p.sum(-1, keepdims=True)
    ctx = np.einsum('bhqk,bhkd->bhqd', p, v)
    ctx = ctx.transpose(0, 2, 1, 3).reshape(B, S_, D_)
    return (ctx @ W_proj.T + b_proj).astype(np.float32)


def kernel(x, attn_mask, W_qkv, W_proj, b_proj):
    x = np.asarray(x, np.float32)
    attn_mask = np.asarray(attn_mask)
    W_qkv = np.asarray(W_qkv, np.float32)
    W_proj = np.asarray(W_proj, np.float32)
    b_proj = np.asarray(b_proj, np.float32)
    B = x.shape[0]
    m = attn_mask[:, 0]
    lens = m[:, -1, :].sum(-1).astype(np.int64)
    pos = np.arange(S)
    causal = pos[:, None] >= pos[None, :]
    structured = bool((lens >= 1).all()) and all(
        np.array_equal(m[b], causal & (pos[None, :] < lens[b])) for b in range(B))
    if not (structured and B == 8 and x.shape == (8, S, D)):
        return _numpy_fallback(x, attn_mask, W_qkv, W_proj, b_proj)
    maxlen = int(lens.max())
    if maxlen not in _RUNNERS:
        nc = build_nc(num_cores=8, maxlen=maxlen)
        _RUNNERS[maxlen] = _build_runner(nc, 8)
    in_maps = [host_prep(x[b], int(lens[b]), W_qkv, W_proj, b_proj)
               for b in range(B)]
    results = _RUNNERS[maxlen](in_maps)
    return np.stack([results[c]["y"] for c in range(8)])


# revision 27
# speedup vs baseline: 2.7805x; 2.7805x over previous
"""Trainium2 Bass kernel: 4-head causal+ragged attention, one sample per core.

bf16 datapath: QKV projection, scores (QK^T with causal fixup via tril matmul),
exp on ScalarE with per-key-block mask bias, AV accumulation with a fused
ones-row denominator, reciprocal on a gathered [4, FILL] tile, and output
projection. PSUM accumulation stays fp32. Host wrapper verifies the mask is
causal & key-length structured, shards one sample per core, and gathers.
Falls back to a pure-numpy reference path for unstructured masks.
"""
import sys
sys.path.insert(0, '/opt/trn_rl_repo')
import numpy as np
import ml_dtypes
import concourse.bacc as bacc
import concourse.mybir as mybir
from concourse.tile import TileContext
from concourse.dve_ops import RECIP_APPROX_FAST_CONSTS, RECIPROCAL_APPROX_FAST

F32 = mybir.dt.float32
F32R = mybir.dt.float32r
BF16 = mybir.dt.bfloat16
EXP = mybir.ActivationFunctionType.Exp

S = 2048
D = 64
H = 4
DH = 16
NKB = S // 128
FILL = 1024
NEG = -1e30


def build_nc(num_cores=8, loop_n=1, maxlen=S, dbg=False):
    kb_max = (int(maxlen) + 127) // 128 - 1  # last key block any sample attends
    nc = bacc.Bacc("TRN2", target_bir_lowering=False, debug=False, num_devices=num_cores)
    QT = nc.dram_tensor("qt", [128, S], BF16, kind="ExternalInput").ap()
    KT = nc.dram_tensor("kt", [128, S], BF16, kind="ExternalInput").ap()
    QT3 = nc.dram_tensor("qt3", [DH, S], BF16, kind="ExternalInput").ap()
    KT3 = nc.dram_tensor("kt3", [DH, S], BF16, kind="ExternalInput").ap()
    V4 = nc.dram_tensor("v4", [128, 2 * S], BF16, kind="ExternalInput").ap()
    MTRIL = nc.dram_tensor("mtril", [128, 128], BF16, kind="ExternalInput").ap()
    KBIAS = nc.dram_tensor("kbias", [128, NKB], F32, kind="ExternalInput").ap()
    WP = nc.dram_tensor("wp", [128, D], BF16, kind="ExternalInput").ap()
    EB4 = nc.dram_tensor("eb4", [H, 128], F32R, kind="ExternalInput").ap()
    Y = nc.dram_tensor("y", [S, D], F32, kind="ExternalOutput").ap()
    if dbg:
        DQK = nc.dram_tensor("d_qk", [128, 2 * S], BF16, kind="ExternalOutput").ap()
        DV = nc.dram_tensor("d_v", [128, 2 * S], BF16, kind="ExternalOutput").ap()
        DCTX = nc.dram_tensor("d_ctx", [128, S], BF16, kind="ExternalOutput").ap()
        DR4 = nc.dram_tensor("d_r4", [H, S], F32, kind="ExternalOutput").ap()
        DZ = nc.dram_tensor("d_z", [128, S], BF16, kind="ExternalOutput").ap()
        DEX = nc.dram_tensor("d_ex", [128, S], BF16, kind="ExternalOutput").ap()

    import contextlib
    with TileContext(nc) as tc, nc.allow_low_precision(
            reason="bf16 datapath intended; accumulation stays fp32 in PSUM"):
        loop_cm = tc.For_i(0, loop_n, 1) if loop_n > 1 else contextlib.nullcontext()
        with loop_cm, \
             tc.sbuf_pool(name="const", bufs=1) as cp, \
             tc.sbuf_pool(name="sb", bufs=1) as sp, \
             tc.sbuf_pool(name="ex", bufs=9) as ep:
            mtril = cp.tile([128, 128], BF16)
            nc.sync.dma_start(out=mtril[:], in_=MTRIL[:])
            kbias = cp.tile([128, NKB], F32)
            nc.sync.dma_start(out=kbias[:], in_=KBIAS[:])
            wp = cp.tile([128, D], BF16)
            nc.sync.dma_start(out=wp[:], in_=WP[:])
            eb4 = cp.tile([H, 128], F32R)
            nc.sync.dma_start(out=eb4[:], in_=EB4[:])

            # host-precomputed projections: heads packed at 32-row stripes;
            # head 3 in base-0 tiles (PE operand base partition must be 0/32/64)
            qT_s = sp.tile([128, S], BF16)
            nc.sync.dma_start(out=qT_s[:], in_=QT[:])
            kT_s = sp.tile([128, S], BF16)
            nc.sync.dma_start(out=kT_s[:], in_=KT[:])
            qT_s3 = sp.tile([DH, S], BF16)
            nc.sync.dma_start(out=qT_s3[:], in_=QT3[:])
            kT_s3 = sp.tile([DH, S], BF16)
            nc.sync.dma_start(out=kT_s3[:], in_=KT3[:])
            # per kb block: 4 head-blocks of 64 cols; head h nonzero only in
            # its 32-col stripe so pair-overlapped AV accumulation adds zeros
            v_s = sp.tile([128, 2 * S], BF16)
            nc.sync.dma_start(out=v_s[:], in_=V4[:, 0:2 * S])

            if dbg:
                nc.sync.dma_start(out=DQK[:, 0:S], in_=qT_s[:])
                nc.sync.dma_start(out=DQK[:, S:2 * S], in_=kT_s[:])
                nc.sync.dma_start(out=DV[:], in_=v_s[:])
            # packed denominators (from pctx rows 0,1,64,65 via ctx_s + DMA
            # partition shift -- the custom recip DVE op misbehaves at
            # partition offsets > 0, so everything runs at offset 0)
            den2 = sp.tile([H, S], BF16)
            den4 = sp.tile([H, S], F32)
            R4g = sp.tile([H, S], F32R)

            with tc.psum_pool(name="ps", bufs=3) as ps, \
                 tc.psum_pool(name="pc", bufs=1) as pc:
                fstate = {}

                # finalize is split in three stages injected into the NEXT
                # q-chunk's unit stream: (a) drain pctx to SBUF + reciprocal
                # right away (frees the single pctx buffer for reuse), (b) the
                # recip broadcast matmul once the PE has other work queued,
                # (c) the output projection once z is surely ready.
                def recip_f32r(out, in_):
                    c = RECIP_APPROX_FAST_CONSTS
                    nc.vector._custom_dve(RECIPROCAL_APPROX_FAST, out=out,
                                          in0=in_, s0=c["s0"], s1=c["s1"],
                                          imm2=c["imm2"])

                def emit_finalize_a(qc, pctx):
                    qlo = qc * FILL
                    ctx_s = sp.tile([128, FILL], BF16, tag="ctxs", bufs=2)
                    nc.vector.tensor_copy(ctx_s[:], pctx[:])
                    nc.sync.dma_start(out=den2[0:2, qlo:qlo + FILL],
                                      in_=ctx_s[0:2, :])
                    nc.sync.dma_start(out=den2[2:4, qlo:qlo + FILL],
                                      in_=ctx_s[64:66, :])
                    nc.vector.tensor_copy(den4[:, qlo:qlo + FILL],
                                          den2[:, qlo:qlo + FILL])
                    recip_f32r(R4g[:, qlo:qlo + FILL],
                               den4[:, qlo:qlo + FILL])
                    if dbg:
                        nc.sync.dma_start(out=DCTX[:, qlo:qlo + FILL], in_=ctx_s[:])
                        nc.sync.dma_start(out=DR4[:, qlo:qlo + FILL],
                                          in_=R4g[:, qlo:qlo + FILL].bitcast(F32))

                    fstate[qc] = ctx_s

                def emit_finalize_b(qc):
                    qlo = qc * FILL
                    ctx_s = fstate[qc]
                    rb = ps.tile([128, FILL], F32, tag="st")
                    for c in range(FILL // 512):
                        lo = c * 512
                        nc.tensor.matmul(rb[:, lo:lo + 512], eb4[:],
                                         R4g[:, qlo + lo:qlo + lo + 512],
                                         start=True, stop=True)
                    rbs = sp.tile([128, FILL], BF16, tag="rbs", bufs=2)
                    nc.vector.tensor_copy(rbs[:], rb[:])
                    z = sp.tile([128, FILL], BF16, tag="z", bufs=2)
                    nc.vector.tensor_mul(z[:], ctx_s[:], rbs[:])
                    if dbg:
                        nc.sync.dma_start(out=DZ[:, qlo:qlo + FILL], in_=z[:])
                    fstate[qc] = z

                def emit_finalize_c(qc):
                    qlo = qc * FILL
                    z = fstate.pop(qc)
                    for t in range(FILL // 128):
                        t0 = qlo + t * 128
                        py = ps.tile([128, D], F32, tag="st")
                        nc.tensor.matmul(py[:], z[:, t * 128:(t + 1) * 128], wp[:],
                                         start=True, stop=True)
                        ys = sp.tile([128, D], F32, tag="ys", bufs=3)
                        nc.vector.tensor_copy(ys[:], py[:])
                        nc.sync.dma_start(out=Y[t0:t0 + 128, :], in_=ys[:])

                nqc = S // FILL
                prev_pctx = None
                for qc in range(nqc):
                    qlo, qhi = qc * FILL, (qc + 1) * FILL
                    if prev_pctx is not None:
                        emit_finalize_a(qc - 1, prev_pctx)
                    pctx = pc.tile([128, FILL], F32, tag="pctx")
                    prev_pctx = pctx
                    kbs = [kb for kb in range(min(qhi // 128, kb_max + 1))]
                    pendings = []  # delayed AV batches: (exs, n0, kb, last)

                    def emit_av_batch(pending):
                        pexs, pn0, pkb, plast = pending
                        q0p = 128 * pkb
                        for ph in (0, 2, 1, 3):  # alternate col groups: LDW overlap
                            p = ph // 2
                            for c in range(FILL // 512):
                                s0 = qlo + c * 512
                                if s0 + 512 <= q0p:
                                    continue
                                b0 = max(s0, q0p) - qlo
                                nc.tensor.matmul(
                                    pctx[64 * p:64 * p + 64, b0:(c + 1) * 512],
                                    v_s[:, 256 * pkb + 64 * ph:256 * pkb + 64 * ph + 64],
                                    pexs[ph][:, b0:(c + 1) * 512],
                                    start=(pkb == 0 and ph % 2 == 0),
                                    stop=(plast and ph % 2 == 1),
                                    skip_group_check=True)

                    for ki, kb in enumerate(kbs):
                        q0 = 128 * kb
                        n0 = max(q0 - qlo, 0)
                        diag = q0 >= qlo
                        exs = []
                        for h in range(H):  # scores back-to-back: one PE mode
                            if h < 3:
                                kT = kT_s[32 * h:32 * h + DH, q0:q0 + 128]
                                qT_f, h_lo = qT_s, h
                            else:
                                kT = kT_s3[:, q0:q0 + 128]
                                qT_f, h_lo = qT_s3, 0
                            st = ps.tile([128, FILL], F32, tag="st")
                            for c in range(FILL // 512):
                                s0 = qlo + c * 512
                                if s0 + 512 <= q0:
                                    continue
                                b0 = max(s0, q0) - qlo
                                nc.tensor.matmul(st[:, b0:(c + 1) * 512], kT,
                                                 qT_f[32 * h_lo:32 * h_lo + DH,
                                                      qlo + b0:s0 + 512],
                                                 start=True, stop=True)
                            ex = ep.tile([128, FILL], BF16, tag="ex")
                            nc.scalar.activation(ex[:, n0:FILL], st[:, n0:FILL],
                                                 EXP, bias=kbias[:, kb:kb + 1],
                                                 scale=0.25)
                            if diag:
                                # causal mask inside the diagonal 128-block:
                                # multiply by 0/1 mask on DVE (cheaper than a
                                # PE matmul that would thrash the tile mode)
                                nc.vector.tensor_mul(ex[:, n0:n0 + 128],
                                                     ex[:, n0:n0 + 128],
                                                     mtril[:])
                            exs.append(ex)
                        pendings.append((exs, n0, kb, kb == kbs[-1]))
                        if len(pendings) > 1:
                            emit_av_batch(pendings.pop(0))
                        # inject previous q-chunk's finalize once this chunk's
                        # pipeline is warm, so PE never idles on the recip chain
                        if qc > 0 and ki == 2:
                            emit_finalize_b(qc - 1)
                        if qc > 0 and ki == 3:
                            emit_finalize_c(qc - 1)
                    for pnd in pendings:
                        emit_av_batch(pnd)
                    pendings = []
                emit_finalize_a(nqc - 1, prev_pctx)
                emit_finalize_b(nqc - 1)
                emit_finalize_c(nqc - 1)
    nc.compile()
    return nc


def host_prep(x_b, lens_b, W_qkv, W_proj, b_proj):
    bf = ml_dtypes.bfloat16
    x_b = np.asarray(x_b, np.float32)
    q = x_b @ W_qkv[0:D].T                      # [S, D]
    k = x_b @ W_qkv[D:2 * D].T
    v = x_b @ W_qkv[2 * D:3 * D].T
    qT = np.zeros((128, S), bf)
    kT = np.zeros((128, S), bf)
    for h in range(H):
        qT[32 * h:32 * h + DH] = q[:, DH * h:DH * h + DH].T.astype(bf)
        kT[32 * h:32 * h + DH] = k[:, DH * h:DH * h + DH].T.astype(bf)
    qT3 = np.ascontiguousarray(qT[96:96 + DH])
    kT3 = np.ascontiguousarray(kT[96:96 + DH])
    # v4: per kb block of 256 cols, head h at 64h..64h+64; within the block:
    # even head: ones col at j=0, dims at j=2..18; odd head: ones at j=1,
    # dims at j=34..50; h0 also carries the bias row at j=19 (ones)
    v4 = np.zeros((128, 2 * S), bf)
    nkb = S // 128
    for kb in range(nkb):
        vb = v[kb * 128:(kb + 1) * 128]         # [128 keys, 64]
        for h in range(H):
            base = 256 * kb + 64 * h
            v4[:, base + h % 2] = 1.0
            jdim = 2 + 32 * (h % 2)
            v4[:, base + jdim:base + jdim + DH] = \
                vb[:, DH * h:DH * h + DH].astype(bf)
        v4[:, 256 * kb + 19] = 1.0
    j = np.arange(128)
    mtril = (j[:, None] <= j[None, :]).astype(bf)  # [key, q]: 1 = attend
    pos = np.arange(S)
    kbias = np.ascontiguousarray(
        np.where((pos < lens_b).reshape(NKB, 128).T, np.float32(0.0),
                 np.float32(NEG)))
    wp = np.zeros((128, D), bf)
    for h in range(H):
        r0 = 64 * (h // 2) + 2 + 32 * (h % 2)
        wp[r0:r0 + DH, :] = W_proj[:, DH * h:DH * h + DH].T.astype(bf)
    wp[19, :] = np.asarray(b_proj, np.float32).astype(bf)
    eb4 = np.zeros((H, 128), np.float32)
    for h in range(H):
        r0 = 64 * (h // 2)
        eb4[h, r0 + h % 2] = 1.0
        d0 = r0 + 2 + 32 * (h % 2)
        eb4[h, d0:d0 + DH] = 1.0
    eb4[0, 19] = 1.0
    return {"qt": qT, "kt": kT, "qt3": qT3, "kt3": kT3, "v4": v4,
            "mtril": mtril, "kbias": kbias, "wp": wp, "eb4": eb4}


_RUNNERS = {}


def _build_runner(nc, n_cores=8):
    import jax
    from jax.sharding import Mesh, PartitionSpec
    from jax.experimental.shard_map import shard_map
    from concourse.bass2jax import (_bass_exec_p, install_neuronx_cc_hook,
                                    partition_id_tensor)
    install_neuronx_cc_hook()
    partition_name = nc.partition_id_tensor.name if nc.partition_id_tensor else None
    in_names, out_names, out_avals, zero_outs = [], [], [], []
    for alloc in nc.m.functions[0].allocations:
        if not isinstance(alloc, mybir.MemoryLocationSet):
            continue
        name = alloc.memorylocations[0].name
        if alloc.kind == "ExternalInput":
            if name != partition_name:
                in_names.append(name)
        elif alloc.kind == "ExternalOutput":
            shape = tuple(alloc.tensor_shape)
            dtype = mybir.dt.np(alloc.dtype)
            out_names.append(name)
            out_avals.append(jax.core.ShapedArray(shape, dtype))
            zero_outs.append(np.zeros(shape, dtype))
    n_params = len(in_names)
    n_outs = len(out_avals)
    all_in_names = list(in_names) + list(out_names)
    if partition_name is not None:
        all_in_names.append(partition_name)
    donate = tuple(range(n_params, n_params + n_outs))

    def _body(*args):
        operands = list(args)
        if partition_name is not None:
            operands.append(partition_id_tensor())
        outs = _bass_exec_p.bind(
            *operands,
            out_avals=tuple(out_avals),
            in_names=tuple(all_in_names),
            out_names=tuple(out_names),
            lowering_input_output_aliases=(),
            sim_require_finite=True,
            sim_require_nnan=True,
            nc=nc,
        )
        return tuple(outs)

    devices = jax.devices()[:n_cores]
    mesh = Mesh(np.asarray(devices), ("core",))
    in_specs = (PartitionSpec("core"),) * (n_params + n_outs)
    out_specs = (PartitionSpec("core"),) * n_outs
    sharded = jax.jit(
        shard_map(_body, mesh=mesh, in_specs=in_specs, out_specs=out_specs,
                  check_rep=False),
        donate_argnums=donate, keep_unused=True)

    def run(in_maps):
        import jax
        per_core = [[np.asarray(m[n]) for n in in_names] for m in in_maps]
        concat_in = [np.concatenate([per_core[c][i] for c in range(n_cores)], axis=0)
                     for i in range(n_params)]
        concat_zeros = [np.zeros((n_cores * z.shape[0], *z.shape[1:]), z.dtype)
                        for z in zero_outs]
        out_arrs = sharded(*concat_in, *concat_zeros)
        jax.block_until_ready(out_arrs)
        return [
            {name: np.asarray(out_arrs[i]).reshape(n_cores, *out_avals[i].shape)[c]
             for i, name in enumerate(out_names)}
            for c in range(n_cores)
        ]
    return run


def _numpy_fallback(x, attn_mask, W_qkv, W_proj, b_proj):
    B, S_, D_ = x.shape
    qkv = x @ W_qkv.T
    qkv = qkv.reshape(B, S_, 3, H, DH).transpose(2, 0, 3, 1, 4)
    q, k, v = qkv[0], qkv[1], qkv[2]
    s = np.einsum('bhqd,bhkd->bhqk', q, k).astype(np.float32) / np.sqrt(DH)
    neg = np.finfo(np.float32).min
    s = np.where(attn_mask, s, neg)
    s = s - s.max(-1, keepdims=True)
    p = np.exp(s)
    p = p / p.sum(-1, keepdims=True)
    ctx = np.einsum('bhqk,bhkd->bhqd', p, v)
    ctx = ctx.transpose(0, 2, 1, 3).reshape(B, S_, D_)
    return (ctx @ W_proj.T + b_proj).astype(np.float32)


def kernel(x, attn_mask, W_qkv, W_proj, b_proj):
    x = np.asarray(x, np.float32)
    attn_mask = np.asarray(attn_mask)
    W_qkv = np.asarray(W_qkv, np.float32)
    W_proj = np.asarray(W_proj, np.float32)
    b_proj = np.asarray(b_proj, np.float32)
    B = x.shape[0]
    m = attn_mask[:, 0]
    lens = m[:, -1, :].sum(-1).astype(np.int64)
    pos = np.arange(S)
    causal = pos[:, None] >= pos[None, :]
    structured = bool((lens >= 1).all()) and all(
        np.array_equal(m[b], causal & (pos[None, :] < lens[b])) for b in range(B))
    if not (structured and B == 8 and x.shape == (8, S, D)):
        return _numpy_fallback(x, attn_mask, W_qkv, W_proj, b_proj)
    maxlen = int(lens.max())
    if maxlen not in _RUNNERS:
        nc = build_nc(num_cores=8, maxlen=maxlen)
        _RUNNERS[maxlen] = _build_runner(nc, 8)
    in_maps = [host_prep(x[b], int(lens[b]), W_qkv, W_proj, b_proj)
               for b in range(B)]
    results = _RUNNERS[maxlen](in_maps)
    return np.stack([results[c]["y"] for c in range(8)])


# revision 29
# speedup vs baseline: 3.2046x; 1.1525x over previous
"""Trainium2 Bass kernel: 4-head causal+ragged attention, one sample per core.

bf16 datapath: QKV projection, scores (QK^T with causal fixup via tril matmul),
exp on ScalarE with per-key-block mask bias, AV accumulation with a fused
ones-row denominator, reciprocal on a gathered [4, FILL] tile, and output
projection. PSUM accumulation stays fp32. Host wrapper verifies the mask is
causal & key-length structured, shards one sample per core, and gathers.
Falls back to a pure-numpy reference path for unstructured masks.
"""
import sys
sys.path.insert(0, '/opt/trn_rl_repo')
import numpy as np
import ml_dtypes
import concourse.bacc as bacc
import concourse.mybir as mybir
from concourse.tile import TileContext
from concourse.dve_ops import RECIP_APPROX_FAST_CONSTS, RECIPROCAL_APPROX_FAST

F32 = mybir.dt.float32
F32R = mybir.dt.float32r
BF16 = mybir.dt.bfloat16
EXP = mybir.ActivationFunctionType.Exp

S = 2048
D = 64
H = 4
DH = 16
NKB = S // 128
FILL = 1024
NEG = -1e30


def build_nc(num_cores=8, loop_n=1, maxlen=S, dbg=False):
    kb_max = (int(maxlen) + 127) // 128 - 1  # last key block any sample attends
    nc = bacc.Bacc("TRN2", target_bir_lowering=False, debug=False, num_devices=num_cores)
    QT = nc.dram_tensor("qt", [128, S], BF16, kind="ExternalInput").ap()
    KT4 = nc.dram_tensor("kt4", [128, 4 * S], BF16, kind="ExternalInput").ap()
    V4 = nc.dram_tensor("v4", [128, 4 * S], BF16, kind="ExternalInput").ap()
    MTRIL = nc.dram_tensor("mtril", [128, 128], BF16, kind="ExternalInput").ap()
    KBIAS = nc.dram_tensor("kbias", [128, NKB], F32, kind="ExternalInput").ap()
    WP = nc.dram_tensor("wp", [128, D], BF16, kind="ExternalInput").ap()
    EB4 = nc.dram_tensor("eb4", [H, 128], F32R, kind="ExternalInput").ap()
    Y = nc.dram_tensor("y", [S, D], F32, kind="ExternalOutput").ap()
    if dbg:
        DQK = nc.dram_tensor("d_qk", [128, 2 * S], BF16, kind="ExternalOutput").ap()
        DV = nc.dram_tensor("d_v", [128, 2 * S], BF16, kind="ExternalOutput").ap()
        DCTX = nc.dram_tensor("d_ctx", [128, S], BF16, kind="ExternalOutput").ap()
        DR4 = nc.dram_tensor("d_r4", [H, S], F32, kind="ExternalOutput").ap()
        DZ = nc.dram_tensor("d_z", [128, S], BF16, kind="ExternalOutput").ap()
        DEX = nc.dram_tensor("d_ex", [128, S], BF16, kind="ExternalOutput").ap()

    import contextlib
    with TileContext(nc) as tc, nc.allow_low_precision(
            reason="bf16 datapath intended; accumulation stays fp32 in PSUM"):
        loop_cm = tc.For_i(0, loop_n, 1) if loop_n > 1 else contextlib.nullcontext()
        with loop_cm, \
             tc.sbuf_pool(name="const", bufs=1) as cp, \
             tc.sbuf_pool(name="sb", bufs=1) as sp, \
             tc.sbuf_pool(name="ex", bufs=9) as ep:
            mtril = cp.tile([128, 128], BF16)
            nc.sync.dma_start(out=mtril[:], in_=MTRIL[:])
            kbias = cp.tile([128, NKB], F32)
            nc.sync.dma_start(out=kbias[:], in_=KBIAS[:])
            wp = cp.tile([128, D], BF16)
            nc.sync.dma_start(out=wp[:], in_=WP[:])
            eb4 = cp.tile([H, 128], F32R)
            nc.sync.dma_start(out=eb4[:], in_=EB4[:])

            # host-precomputed projections. kT4: per (kb, h) a full 128-row
            # lhsT block with only head h's 16-row stripe nonzero -- keeps
            # every score matmul in (128,128) tile mode (no PE mode-switch
            # drains) and removes the head-3 base-partition special case.
            # v4: per (kb, h) a 128-col block targeting all pctx rows.
            qT_s = sp.tile([128, S], BF16)
            nc.sync.dma_start(out=qT_s[:], in_=QT[:])
            kT4 = sp.tile([128, 4 * S], BF16)
            nc.sync.dma_start(out=kT4[:], in_=KT4[:])
            v_s = sp.tile([128, 4 * S], BF16)
            nc.sync.dma_start(out=v_s[:], in_=V4[:])

            if dbg:
                nc.sync.dma_start(out=DQK[:, 0:S], in_=qT_s[:])
                nc.sync.dma_start(out=DQK[:, S:2 * S], in_=kT_s[:])
                nc.sync.dma_start(out=DV[:], in_=v_s[:])
            # packed denominators (from pctx rows 0,1,64,65 via ctx_s + DMA
            # partition shift -- the custom recip DVE op misbehaves at
            # partition offsets > 0, so everything runs at offset 0)
            den2 = sp.tile([H, S], BF16)
            den4 = sp.tile([H, S], F32)
            R4g = sp.tile([H, S], F32R)

            with tc.psum_pool(name="ps", bufs=3) as ps, \
                 tc.psum_pool(name="pc", bufs=1) as pc:
                fstate = {}

                # finalize is split in three stages injected into the NEXT
                # q-chunk's unit stream: (a) drain pctx to SBUF + reciprocal
                # right away (frees the single pctx buffer for reuse), (b) the
                # recip broadcast matmul once the PE has other work queued,
                # (c) the output projection once z is surely ready.
                def recip_f32r(out, in_):
                    c = RECIP_APPROX_FAST_CONSTS
                    nc.vector._custom_dve(RECIPROCAL_APPROX_FAST, out=out,
                                          in0=in_, s0=c["s0"], s1=c["s1"],
                                          imm2=c["imm2"])

                def emit_finalize_a(qc, pctx):
                    qlo = qc * FILL
                    ctx_s = sp.tile([128, FILL], BF16, tag="ctxs", bufs=2)
                    nc.vector.tensor_copy(ctx_s[:], pctx[:])
                    nc.sync.dma_start(out=den2[0:2, qlo:qlo + FILL],
                                      in_=ctx_s[0:2, :])
                    nc.sync.dma_start(out=den2[2:4, qlo:qlo + FILL],
                                      in_=ctx_s[64:66, :])
                    nc.vector.tensor_copy(den4[:, qlo:qlo + FILL],
                                          den2[:, qlo:qlo + FILL])
                    recip_f32r(R4g[:, qlo:qlo + FILL],
                               den4[:, qlo:qlo + FILL])
                    if dbg:
                        nc.sync.dma_start(out=DCTX[:, qlo:qlo + FILL], in_=ctx_s[:])
                        nc.sync.dma_start(out=DR4[:, qlo:qlo + FILL],
                                          in_=R4g[:, qlo:qlo + FILL].bitcast(F32))

                    fstate[qc] = ctx_s

                def emit_finalize_b(qc):
                    qlo = qc * FILL
                    ctx_s = fstate[qc]
                    rb = ps.tile([128, FILL], F32, tag="st")
                    for c in range(FILL // 512):
                        lo = c * 512
                        nc.tensor.matmul(rb[:, lo:lo + 512], eb4[:],
                                         R4g[:, qlo + lo:qlo + lo + 512],
                                         start=True, stop=True)
                    rbs = sp.tile([128, FILL], BF16, tag="rbs", bufs=2)
                    nc.vector.tensor_copy(rbs[:], rb[:])
                    z = sp.tile([128, FILL], BF16, tag="z", bufs=2)
                    nc.vector.tensor_mul(z[:], ctx_s[:], rbs[:])
                    if dbg:
                        nc.sync.dma_start(out=DZ[:, qlo:qlo + FILL], in_=z[:])
                    fstate[qc] = z

                def emit_finalize_c(qc):
                    qlo = qc * FILL
                    z = fstate.pop(qc)
                    for t in range(FILL // 128):
                        t0 = qlo + t * 128
                        py = ps.tile([128, D], F32, tag="st")
                        nc.tensor.matmul(py[:], z[:, t * 128:(t + 1) * 128], wp[:],
                                         start=True, stop=True)
                        ys = sp.tile([128, D], F32, tag="ys", bufs=3)
                        nc.vector.tensor_copy(ys[:], py[:])
                        nc.sync.dma_start(out=Y[t0:t0 + 128, :], in_=ys[:])

                nqc = S // FILL
                prev_pctx = None
                for qc in range(nqc):
                    qlo, qhi = qc * FILL, (qc + 1) * FILL
                    if prev_pctx is not None:
                        emit_finalize_a(qc - 1, prev_pctx)
                    pctx = pc.tile([128, FILL], F32, tag="pctx")
                    prev_pctx = pctx
                    kbs = [kb for kb in range(min(qhi // 128, kb_max + 1))]
                    pendings = []  # delayed AV batches: (exs, n0, kb, last)

                    def emit_av_batch(pending):
                        pexs, pn0, pkb, plast = pending
                        q0p = 128 * pkb
                        for ph in range(H):
                            for c in range(FILL // 512):
                                s0 = qlo + c * 512
                                if s0 + 512 <= q0p:
                                    continue
                                b0 = max(s0, q0p) - qlo
                                nc.tensor.matmul(
                                    pctx[:, b0:(c + 1) * 512],
                                    v_s[:, 512 * pkb + 128 * ph:512 * pkb + 128 * ph + 128],
                                    pexs[ph][:, b0:(c + 1) * 512],
                                    start=(pkb == 0 and ph == 0),
                                    stop=(plast and ph == H - 1),
                                    skip_group_check=True)

                    for ki, kb in enumerate(kbs):
                        q0 = 128 * kb
                        n0 = max(q0 - qlo, 0)
                        diag = q0 >= qlo
                        exs = []
                        for h in range(H):  # scores back-to-back: one PE mode
                            kT = kT4[:, 512 * kb + 128 * h:512 * kb + 128 * h + 128]
                            st = ps.tile([128, FILL], F32, tag="st")
                            for c in range(FILL // 512):
                                s0 = qlo + c * 512
                                if s0 + 512 <= q0:
                                    continue
                                b0 = max(s0, q0) - qlo
                                nc.tensor.matmul(st[:, b0:(c + 1) * 512], kT,
                                                 qT_s[:, qlo + b0:s0 + 512],
                                                 start=True, stop=True)
                            ex = ep.tile([128, FILL], BF16, tag="ex")
                            nc.scalar.activation(ex[:, n0:FILL], st[:, n0:FILL],
                                                 EXP, bias=kbias[:, kb:kb + 1],
                                                 scale=0.25)
                            if diag:
                                # causal mask inside the diagonal 128-block:
                                # multiply by 0/1 mask on DVE (cheaper than a
                                # PE matmul that would thrash the tile mode)
                                nc.vector.tensor_mul(ex[:, n0:n0 + 128],
                                                     ex[:, n0:n0 + 128],
                                                     mtril[:])
                            exs.append(ex)
                        pendings.append((exs, n0, kb, kb == kbs[-1]))
                        if len(pendings) > 1:
                            emit_av_batch(pendings.pop(0))
                        # inject previous q-chunk's finalize once this chunk's
                        # pipeline is warm, so PE never idles on the recip chain
                        if qc > 0 and ki == 2:
                            emit_finalize_b(qc - 1)
                        if qc > 0 and ki == 3:
                            emit_finalize_c(qc - 1)
                    for pnd in pendings:
                        emit_av_batch(pnd)
                    pendings = []
                emit_finalize_a(nqc - 1, prev_pctx)
                emit_finalize_b(nqc - 1)
                emit_finalize_c(nqc - 1)
    nc.compile()
    return nc


def host_prep(x_b, lens_b, W_qkv, W_proj, b_proj):
    bf = ml_dtypes.bfloat16
    x_b = np.asarray(x_b, np.float32)
    q = x_b @ W_qkv[0:D].T                      # [S, D]
    k = x_b @ W_qkv[D:2 * D].T
    v = x_b @ W_qkv[2 * D:3 * D].T
    qT = np.zeros((128, S), bf)
    for h in range(H):
        qT[32 * h:32 * h + DH] = q[:, DH * h:DH * h + DH].T.astype(bf)
    # kt4: per (kb, h) a [128, 128] lhsT block; only head h's stripe rows
    # are nonzero so contraction against the full qT zeroes other heads
    kbf = np.zeros((128, S), np.float32)
    for h in range(H):
        kbf[32 * h:32 * h + DH] = k[:, DH * h:DH * h + DH].T
    kt4 = np.zeros((128, 4 * S), bf)
    nkb = S // 128
    for kb in range(nkb):
        for h in range(H):
            blk = np.zeros((128, 128), np.float32)
            blk[32 * h:32 * h + DH] = kbf[32 * h:32 * h + DH,
                                          kb * 128:(kb + 1) * 128]
            kt4[:, 512 * kb + 128 * h:512 * kb + 128 * h + 128] = blk.astype(bf)
    # v4: per (kb, h) a [128keys, 128 featcols] block: denominator ones col at
    # 64*(h//2)+h%2, dims at 64*(h//2)+2+32*(h%2).., h0 extra ones col at 19
    # (carries b_proj through the z row); zero elsewhere so the full-width
    # accumulation adds zeros outside head h's features
    v4 = np.zeros((128, 4 * S), bf)
    for kb in range(nkb):
        vb = v[kb * 128:(kb + 1) * 128]
        for h in range(H):
            base = 512 * kb + 128 * h
            r0 = 64 * (h // 2)
            v4[:, base + r0 + h % 2] = 1.0
            jdim = r0 + 2 + 32 * (h % 2)
            v4[:, base + jdim:base + jdim + DH] = \
                vb[:, DH * h:DH * h + DH].astype(bf)
            if h == 0:
                v4[:, base + 19] = 1.0
    j = np.arange(128)
    mtril = (j[:, None] <= j[None, :]).astype(bf)  # [key, q]: 1 = attend
    pos = np.arange(S)
    kbias = np.ascontiguousarray(
        np.where((pos < lens_b).reshape(NKB, 128).T, np.float32(0.0),
                 np.float32(NEG)))
    wp = np.zeros((128, D), bf)
    for h in range(H):
        r0 = 64 * (h // 2) + 2 + 32 * (h % 2)
        wp[r0:r0 + DH, :] = W_proj[:, DH * h:DH * h + DH].T.astype(bf)
    wp[19, :] = np.asarray(b_proj, np.float32).astype(bf)
    eb4 = np.zeros((H, 128), np.float32)
    for h in range(H):
        r0 = 64 * (h // 2)
        eb4[h, r0 + h % 2] = 1.0
        d0 = r0 + 2 + 32 * (h % 2)
        eb4[h, d0:d0 + DH] = 1.0
    eb4[0, 19] = 1.0
    return {"qt": qT, "kt4": kt4, "v4": v4,
            "mtril": mtril, "kbias": kbias, "wp": wp, "eb4": eb4}


_RUNNERS = {}


def _build_runner(nc, n_cores=8):
    import jax
    from jax.sharding import Mesh, PartitionSpec
    from jax.experimental.shard_map import shard_map
    from concourse.bass2jax import (_bass_exec_p, install_neuronx_cc_hook,
                                    partition_id_tensor)
    install_neuronx_cc_hook()
    partition_name = nc.partition_id_tensor.name if nc.partition_id_tensor else None
    in_names, out_names, out_avals, zero_outs = [], [], [], []
    for alloc in nc.m.functions[0].allocations:
        if not isinstance(alloc, mybir.MemoryLocationSet):
            continue
        name = alloc.memorylocations[0].name
        if alloc.kind == "ExternalInput":
            if name != partition_name:
                in_names.append(name)
        elif alloc.kind == "ExternalOutput":
            shape = tuple(alloc.tensor_shape)
            dtype = mybir.dt.np(alloc.dtype)
            out_names.append(name)
            out_avals.append(jax.core.ShapedArray(shape, dtype))
            zero_outs.append(np.zeros(shape, dtype))
    n_params = len(in_names)
    n_outs = len(out_avals)
    all_in_names = list(in_names) + list(out_names)
    if partition_name is not None:
        all_in_names.append(partition_name)
    donate = tuple(range(n_params, n_params + n_outs))

    def _body(*args):
        operands = list(args)
        if partition_name is not None:
            operands.append(partition_id_tensor())
        outs = _bass_exec_p.bind(
            *operands,
            out_avals=tuple(out_avals),
            in_names=tuple(all_in_names),
            out_names=tuple(out_names),
            lowering_input_output_aliases=(),
            sim_require_finite=True,
            sim_require_nnan=True,
            nc=nc,
        )
        return tuple(outs)

    devices = jax.devices()[:n_cores]
    mesh = Mesh(np.asarray(devices), ("core",))
    in_specs = (PartitionSpec("core"),) * (n_params + n_outs)
    out_specs = (PartitionSpec("core"),) * n_outs
    sharded = jax.jit(
        shard_map(_body, mesh=mesh, in_specs=in_specs, out_specs=out_specs,
                  check_rep=False),
        donate_argnums=donate, keep_unused=True)

    def run(in_maps):
        import jax
        per_core = [[np.asarray(m[n]) for n in in_names] for m in in_maps]
        concat_in = [np.concatenate([per_core[c][i] for c in range(n_cores)], axis=0)
                     for i in range(n_params)]
        concat_zeros = [np.zeros((n_cores * z.shape[0], *z.shape[1:]), z.dtype)
                        for z in zero_outs]
        out_arrs = sharded(*concat_in, *concat_zeros)
        jax.block_until_ready(out_arrs)
        return [
            {name: np.asarray(out_arrs[i]).reshape(n_cores, *out_avals[i].shape)[c]
             for i, name in enumerate(out_names)}
            for c in range(n_cores)
        ]
    return run


def _numpy_fallback(x, attn_mask, W_qkv, W_proj, b_proj):
    B, S_, D_ = x.shape
    qkv = x @ W_qkv.T
    qkv = qkv.reshape(B, S_, 3, H, DH).transpose(2, 0, 3, 1, 4)
    q, k, v = qkv[0], qkv[1], qkv[2]
    s = np.einsum('bhqd,bhkd->bhqk', q, k).astype(np.float32) / np.sqrt(DH)
    neg = np.finfo(np.float32).min
    s = np.where(attn_mask, s, neg)
    s = s - s.max(-1, keepdims=True)
    p = np.exp(s)
    p = p / p.sum(-1, keepdims=True)
    ctx = np.einsum('bhqk,bhkd->bhqd', p, v)
    ctx = ctx.transpose(0, 2, 1, 3).reshape(B, S_, D_)
    return (ctx @ W_proj.T + b_proj).astype(np.float32)


def kernel(x, attn_mask, W_qkv, W_proj, b_proj):
    x = np.asarray(x, np.float32)
    attn_mask = np.asarray(attn_mask)
    W_qkv = np.asarray(W_qkv, np.float32)
    W_proj = np.asarray(W_proj, np.float32)
    b_proj = np.asarray(b_proj, np.float32)
    B = x.shape[0]
    m = attn_mask[:, 0]
    lens = m[:, -1, :].sum(-1).astype(np.int64)
    pos = np.arange(S)
    causal = pos[:, None] >= pos[None, :]
    structured = bool((lens >= 1).all()) and all(
        np.array_equal(m[b], causal & (pos[None, :] < lens[b])) for b in range(B))
    if not (structured and B == 8 and x.shape == (8, S, D)):
        return _numpy_fallback(x, attn_mask, W_qkv, W_proj, b_proj)
    maxlen = int(lens.max())
    if maxlen not in _RUNNERS:
        nc = build_nc(num_cores=8, maxlen=maxlen)
        _RUNNERS[maxlen] = _build_runner(nc, 8)
    in_maps = [host_prep(x[b], int(lens[b]), W_qkv, W_proj, b_proj)
               for b in range(B)]
    results = _RUNNERS[maxlen](in_maps)
    return np.stack([results[c]["y"] for c in range(8)])


# revision 30
# speedup vs baseline: 3.2270x; 1.0070x over previous
"""Trainium2 Bass kernel: 4-head causal+ragged attention, one sample per core.

bf16 datapath: QKV projection, scores (QK^T with causal fixup via tril matmul),
exp on ScalarE with per-key-block mask bias, AV accumulation with a fused
ones-row denominator, reciprocal on a gathered [4, FILL] tile, and output
projection. PSUM accumulation stays fp32. Host wrapper verifies the mask is
causal & key-length structured, shards one sample per core, and gathers.
Falls back to a pure-numpy reference path for unstructured masks.
"""
import sys
sys.path.insert(0, '/opt/trn_rl_repo')
import numpy as np
import ml_dtypes
import concourse.bacc as bacc
import concourse.mybir as mybir
from concourse.tile import TileContext
from concourse.dve_ops import RECIP_APPROX_FAST_CONSTS, RECIPROCAL_APPROX_FAST

F32 = mybir.dt.float32
F32R = mybir.dt.float32r
BF16 = mybir.dt.bfloat16
EXP = mybir.ActivationFunctionType.Exp

S = 2048
D = 64
H = 4
DH = 16
NKB = S // 128
FILL = 1024
NEG = -1e30


def build_nc(num_cores=8, loop_n=1, maxlen=S, dbg=False):
    kb_max = (int(maxlen) + 127) // 128 - 1  # last key block any sample attends
    nc = bacc.Bacc("TRN2", target_bir_lowering=False, debug=False, num_devices=num_cores)
    QT = nc.dram_tensor("qt", [128, S], BF16, kind="ExternalInput").ap()
    KT4 = nc.dram_tensor("kt4", [128, 4 * S], BF16, kind="ExternalInput").ap()
    V4 = nc.dram_tensor("v4", [128, 4 * S], BF16, kind="ExternalInput").ap()
    MTRIL = nc.dram_tensor("mtril", [128, 128], BF16, kind="ExternalInput").ap()
    KBIAS = nc.dram_tensor("kbias", [128, NKB], F32, kind="ExternalInput").ap()
    WP = nc.dram_tensor("wp", [128, D], BF16, kind="ExternalInput").ap()
    EB4 = nc.dram_tensor("eb4", [H, 128], F32R, kind="ExternalInput").ap()
    Y = nc.dram_tensor("y", [S, D], F32, kind="ExternalOutput").ap()
    if dbg:
        DQK = nc.dram_tensor("d_qk", [128, 2 * S], BF16, kind="ExternalOutput").ap()
        DV = nc.dram_tensor("d_v", [128, 2 * S], BF16, kind="ExternalOutput").ap()
        DCTX = nc.dram_tensor("d_ctx", [128, S], BF16, kind="ExternalOutput").ap()
        DR4 = nc.dram_tensor("d_r4", [H, S], F32, kind="ExternalOutput").ap()
        DZ = nc.dram_tensor("d_z", [128, S], BF16, kind="ExternalOutput").ap()
        DEX = nc.dram_tensor("d_ex", [128, S], BF16, kind="ExternalOutput").ap()

    import contextlib
    with TileContext(nc) as tc, nc.allow_low_precision(
            reason="bf16 datapath intended; accumulation stays fp32 in PSUM"):
        loop_cm = tc.For_i(0, loop_n, 1) if loop_n > 1 else contextlib.nullcontext()
        with loop_cm, \
             tc.sbuf_pool(name="const", bufs=1) as cp, \
             tc.sbuf_pool(name="sb", bufs=1) as sp, \
             tc.sbuf_pool(name="ex", bufs=12) as ep:
            mtril = cp.tile([128, 128], BF16)
            nc.sync.dma_start(out=mtril[:], in_=MTRIL[:])
            kbias = cp.tile([128, NKB], F32)
            nc.sync.dma_start(out=kbias[:], in_=KBIAS[:])
            wp = cp.tile([128, D], BF16)
            nc.sync.dma_start(out=wp[:], in_=WP[:])
            eb4 = cp.tile([H, 128], F32R)
            nc.sync.dma_start(out=eb4[:], in_=EB4[:])

            # host-precomputed projections. kT4: per (kb, h) a full 128-row
            # lhsT block with only head h's 16-row stripe nonzero -- keeps
            # every score matmul in (128,128) tile mode (no PE mode-switch
            # drains) and removes the head-3 base-partition special case.
            # v4: per (kb, h) a 128-col block targeting all pctx rows.
            qT_s = sp.tile([128, S], BF16)
            nc.sync.dma_start(out=qT_s[:], in_=QT[:])
            kT4 = sp.tile([128, 4 * S], BF16)
            nc.sync.dma_start(out=kT4[:], in_=KT4[:])
            v_s = sp.tile([128, 4 * S], BF16)
            nc.sync.dma_start(out=v_s[:], in_=V4[:])

            if dbg:
                nc.sync.dma_start(out=DQK[:, 0:S], in_=qT_s[:])
                nc.sync.dma_start(out=DQK[:, S:2 * S], in_=kT_s[:])
                nc.sync.dma_start(out=DV[:], in_=v_s[:])
            # packed denominators (from pctx rows 0,1,64,65 via ctx_s + DMA
            # partition shift -- the custom recip DVE op misbehaves at
            # partition offsets > 0, so everything runs at offset 0)
            den2 = sp.tile([H, S], BF16)
            den4 = sp.tile([H, S], F32)
            R4g = sp.tile([H, S], F32R)

            with tc.psum_pool(name="ps", bufs=3) as ps, \
                 tc.psum_pool(name="pc", bufs=1) as pc:
                fstate = {}

                # finalize is split in three stages injected into the NEXT
                # q-chunk's unit stream: (a) drain pctx to SBUF + reciprocal
                # right away (frees the single pctx buffer for reuse), (b) the
                # recip broadcast matmul once the PE has other work queued,
                # (c) the output projection once z is surely ready.
                def recip_f32r(out, in_):
                    c = RECIP_APPROX_FAST_CONSTS
                    nc.vector._custom_dve(RECIPROCAL_APPROX_FAST, out=out,
                                          in0=in_, s0=c["s0"], s1=c["s1"],
                                          imm2=c["imm2"])

                def emit_finalize_a(qc, pctx):
                    qlo = qc * FILL
                    ctx_s = sp.tile([128, FILL], BF16, tag="ctxs", bufs=2)
                    nc.vector.tensor_copy(ctx_s[:], pctx[:])
                    nc.sync.dma_start(out=den2[0:2, qlo:qlo + FILL],
                                      in_=ctx_s[0:2, :])
                    nc.sync.dma_start(out=den2[2:4, qlo:qlo + FILL],
                                      in_=ctx_s[64:66, :])
                    nc.vector.tensor_copy(den4[:, qlo:qlo + FILL],
                                          den2[:, qlo:qlo + FILL])
                    recip_f32r(R4g[:, qlo:qlo + FILL],
                               den4[:, qlo:qlo + FILL])
                    if dbg:
                        nc.sync.dma_start(out=DCTX[:, qlo:qlo + FILL], in_=ctx_s[:])
                        nc.sync.dma_start(out=DR4[:, qlo:qlo + FILL],
                                          in_=R4g[:, qlo:qlo + FILL].bitcast(F32))

                    fstate[qc] = ctx_s

                def emit_finalize_b(qc):
                    qlo = qc * FILL
                    ctx_s = fstate[qc]
                    rb = ps.tile([128, FILL], F32, tag="st")
                    for c in range(FILL // 512):
                        lo = c * 512
                        nc.tensor.matmul(rb[:, lo:lo + 512], eb4[:],
                                         R4g[:, qlo + lo:qlo + lo + 512],
                                         start=True, stop=True)
                    rbs = sp.tile([128, FILL], BF16, tag="rbs", bufs=2)
                    nc.vector.tensor_copy(rbs[:], rb[:])
                    z = sp.tile([128, FILL], BF16, tag="z", bufs=2)
                    nc.vector.tensor_mul(z[:], ctx_s[:], rbs[:])
                    if dbg:
                        nc.sync.dma_start(out=DZ[:, qlo:qlo + FILL], in_=z[:])
                    fstate[qc] = z

                def emit_finalize_c(qc):
                    qlo = qc * FILL
                    z = fstate.pop(qc)
                    for t in range(FILL // 128):
                        t0 = qlo + t * 128
                        py = ps.tile([128, D], F32, tag="st")
                        nc.tensor.matmul(py[:], z[:, t * 128:(t + 1) * 128], wp[:],
                                         start=True, stop=True)
                        ys = sp.tile([128, D], F32, tag="ys", bufs=3)
                        nc.vector.tensor_copy(ys[:], py[:])
                        nc.sync.dma_start(out=Y[t0:t0 + 128, :], in_=ys[:])

                nqc = S // FILL
                prev_pctx = None
                for qc in range(nqc):
                    qlo, qhi = qc * FILL, (qc + 1) * FILL
                    if prev_pctx is not None:
                        emit_finalize_a(qc - 1, prev_pctx)
                    pctx = pc.tile([128, FILL], F32, tag="pctx")
                    prev_pctx = pctx
                    kbs = [kb for kb in range(min(qhi // 128, kb_max + 1))]
                    pendings = []  # delayed AV batches: (exs, n0, kb, last)

                    def emit_av_batch(pending):
                        pexs, pn0, pkb, plast = pending
                        q0p = 128 * pkb
                        for ph in range(H):
                            for c in range(FILL // 512):
                                s0 = qlo + c * 512
                                if s0 + 512 <= q0p:
                                    continue
                                b0 = max(s0, q0p) - qlo
                                nc.tensor.matmul(
                                    pctx[:, b0:(c + 1) * 512],
                                    v_s[:, 512 * pkb + 128 * ph:512 * pkb + 128 * ph + 128],
                                    pexs[ph][:, b0:(c + 1) * 512],
                                    start=(pkb == 0 and ph == 0),
                                    stop=(plast and ph == H - 1),
                                    skip_group_check=True)

                    for ki, kb in enumerate(kbs):
                        q0 = 128 * kb
                        n0 = max(q0 - qlo, 0)
                        diag = q0 >= qlo
                        exs = []
                        for h in range(H):  # scores back-to-back: one PE mode
                            kT = kT4[:, 512 * kb + 128 * h:512 * kb + 128 * h + 128]
                            st = ps.tile([128, FILL], F32, tag="st")
                            for c in range(FILL // 512):
                                s0 = qlo + c * 512
                                if s0 + 512 <= q0:
                                    continue
                                b0 = max(s0, q0) - qlo
                                nc.tensor.matmul(st[:, b0:(c + 1) * 512], kT,
                                                 qT_s[:, qlo + b0:s0 + 512],
                                                 start=True, stop=True)
                            ex = ep.tile([128, FILL], BF16, tag="ex")
                            nc.scalar.activation(ex[:, n0:FILL], st[:, n0:FILL],
                                                 EXP, bias=kbias[:, kb:kb + 1],
                                                 scale=0.25)
                            if diag:
                                # causal mask inside the diagonal 128-block:
                                # multiply by 0/1 mask on DVE (cheaper than a
                                # PE matmul that would thrash the tile mode)
                                nc.vector.tensor_mul(ex[:, n0:n0 + 128],
                                                     ex[:, n0:n0 + 128],
                                                     mtril[:])
                            exs.append(ex)
                        pendings.append((exs, n0, kb, kb == kbs[-1]))
                        if len(pendings) > 1:
                            emit_av_batch(pendings.pop(0))
                        # inject previous q-chunk's finalize once this chunk's
                        # pipeline is warm, so PE never idles on the recip chain
                        if qc > 0 and ki == 2:
                            emit_finalize_b(qc - 1)
                        if qc > 0 and ki == 3:
                            emit_finalize_c(qc - 1)
                    for pnd in pendings:
                        emit_av_batch(pnd)
                    pendings = []
                emit_finalize_a(nqc - 1, prev_pctx)
                emit_finalize_b(nqc - 1)
                emit_finalize_c(nqc - 1)
    nc.compile()
    return nc


def host_prep(x_b, lens_b, W_qkv, W_proj, b_proj):
    bf = ml_dtypes.bfloat16
    x_b = np.asarray(x_b, np.float32)
    q = x_b @ W_qkv[0:D].T                      # [S, D]
    k = x_b @ W_qkv[D:2 * D].T
    v = x_b @ W_qkv[2 * D:3 * D].T
    qT = np.zeros((128, S), bf)
    for h in range(H):
        qT[32 * h:32 * h + DH] = q[:, DH * h:DH * h + DH].T.astype(bf)
    # kt4: per (kb, h) a [128, 128] lhsT block; only head h's stripe rows
    # are nonzero so contraction against the full qT zeroes other heads
    kbf = np.zeros((128, S), np.float32)
    for h in range(H):
        kbf[32 * h:32 * h + DH] = k[:, DH * h:DH * h + DH].T
    kt4 = np.zeros((128, 4 * S), bf)
    nkb = S // 128
    for kb in range(nkb):
        for h in range(H):
            blk = np.zeros((128, 128), np.float32)
            blk[32 * h:32 * h + DH] = kbf[32 * h:32 * h + DH,
                                          kb * 128:(kb + 1) * 128]
            kt4[:, 512 * kb + 128 * h:512 * kb + 128 * h + 128] = blk.astype(bf)
    # v4: per (kb, h) a [128keys, 128 featcols] block: denominator ones col at
    # 64*(h//2)+h%2, dims at 64*(h//2)+2+32*(h%2).., h0 extra ones col at 19
    # (carries b_proj through the z row); zero elsewhere so the full-width
    # accumulation adds zeros outside head h's features
    v4 = np.zeros((128, 4 * S), bf)
    for kb in range(nkb):
        vb = v[kb * 128:(kb + 1) * 128]
        for h in range(H):
            base = 512 * kb + 128 * h
            r0 = 64 * (h // 2)
            v4[:, base + r0 + h % 2] = 1.0
            jdim = r0 + 2 + 32 * (h % 2)
            v4[:, base + jdim:base + jdim + DH] = \
                vb[:, DH * h:DH * h + DH].astype(bf)
            if h == 0:
                v4[:, base + 19] = 1.0
    j = np.arange(128)
    mtril = (j[:, None] <= j[None, :]).astype(bf)  # [key, q]: 1 = attend
    pos = np.arange(S)
    kbias = np.ascontiguousarray(
        np.where((pos < lens_b).reshape(NKB, 128).T, np.float32(0.0),
                 np.float32(NEG)))
    wp = np.zeros((128, D), bf)
    for h in range(H):
        r0 = 64 * (h // 2) + 2 + 32 * (h % 2)
        wp[r0:r0 + DH, :] = W_proj[:, DH * h:DH * h + DH].T.astype(bf)
    wp[19, :] = np.asarray(b_proj, np.float32).astype(bf)
    eb4 = np.zeros((H, 128), np.float32)
    for h in range(H):
        r0 = 64 * (h // 2)
        eb4[h, r0 + h % 2] = 1.0
        d0 = r0 + 2 + 32 * (h % 2)
        eb4[h, d0:d0 + DH] = 1.0
    eb4[0, 19] = 1.0
    return {"qt": qT, "kt4": kt4, "v4": v4,
            "mtril": mtril, "kbias": kbias, "wp": wp, "eb4": eb4}


_RUNNERS = {}


def _build_runner(nc, n_cores=8):
    import jax
    from jax.sharding import Mesh, PartitionSpec
    from jax.experimental.shard_map import shard_map
    from concourse.bass2jax import (_bass_exec_p, install_neuronx_cc_hook,
                                    partition_id_tensor)
    install_neuronx_cc_hook()
    partition_name = nc.partition_id_tensor.name if nc.partition_id_tensor else None
    in_names, out_names, out_avals, zero_outs = [], [], [], []
    for alloc in nc.m.functions[0].allocations:
        if not isinstance(alloc, mybir.MemoryLocationSet):
            continue
        name = alloc.memorylocations[0].name
        if alloc.kind == "ExternalInput":
            if name != partition_name:
                in_names.append(name)
        elif alloc.kind == "ExternalOutput":
            shape = tuple(alloc.tensor_shape)
            dtype = mybir.dt.np(alloc.dtype)
            out_names.append(name)
            out_avals.append(jax.core.ShapedArray(shape, dtype))
            zero_outs.append(np.zeros(shape, dtype))
    n_params = len(in_names)
    n_outs = len(out_avals)
    all_in_names = list(in_names) + list(out_names)
    if partition_name is not None:
        all_in_names.append(partition_name)
    donate = tuple(range(n_params, n_params + n_outs))

    def _body(*args):
        operands = list(args)
        if partition_name is not None:
            operands.append(partition_id_tensor())
        outs = _bass_exec_p.bind(
            *operands,
            out_avals=tuple(out_avals),
            in_names=tuple(all_in_names),
            out_names=tuple(out_names),
            lowering_input_output_aliases=(),
            sim_require_finite=True,
            sim_require_nnan=True,
            nc=nc,
        )
        return tuple(outs)

    devices = jax.devices()[:n_cores]
    mesh = Mesh(np.asarray(devices), ("core",))
    in_specs = (PartitionSpec("core"),) * (n_params + n_outs)
    out_specs = (PartitionSpec("core"),) * n_outs
    sharded = jax.jit(
        shard_map(_body, mesh=mesh, in_specs=in_specs, out_specs=out_specs,
                  check_rep=False),
        donate_argnums=donate, keep_unused=True)

    def run(in_maps):
        import jax
        per_core = [[np.asarray(m[n]) for n in in_names] for m in in_maps]
        concat_in = [np.concatenate([per_core[c][i] for c in range(n_cores)], axis=0)
                     for i in range(n_params)]
        concat_zeros = [np.zeros((n_cores * z.shape[0], *z.shape[1:]), z.dtype)
                        for z in zero_outs]
        out_arrs = sharded(*concat_in, *concat_zeros)
        jax.block_until_ready(out_arrs)
        return [
            {name: np.asarray(out_arrs[i]).reshape(n_cores, *out_avals[i].shape)[c]
             for i, name in enumerate(out_names)}
            for c in range(n_cores)
        ]
    return run


def _numpy_fallback(x, attn_mask, W_qkv, W_proj, b_proj):
    B, S_, D_ = x.shape
    qkv = x @ W_qkv.T
    qkv = qkv.reshape(B, S_, 3, H, DH).transpose(2, 0, 3, 1, 4)
    q, k, v = qkv[0], qkv[1], qkv[2]
    s = np.einsum('bhqd,bhkd->bhqk', q, k).astype(np.float32) / np.sqrt(DH)
    neg = np.finfo(np.float32).min
    s = np.where(attn_mask, s, neg)
    s = s - s.max(-1, keepdims=True)
    p = np.exp(s)
    p = p / p.sum(-1, keepdims=True)
    ctx = np.einsum('bhqk,bhkd->bhqd', p, v)
    ctx = ctx.transpose(0, 2, 1, 3).reshape(B, S_, D_)
    return (ctx @ W_proj.T + b_proj).astype(np.float32)


def kernel(x, attn_mask, W_qkv, W_proj, b_proj):
    x = np.asarray(x, np.float32)
    attn_mask = np.asarray(attn_mask)
    W_qkv = np.asarray(W_qkv, np.float32)
    W_proj = np.asarray(W_proj, np.float32)
    b_proj = np.asarray(b_proj, np.float32)
    B = x.shape[0]
    m = attn_mask[:, 0]
    lens = m[:, -1, :].sum(-1).astype(np.int64)
    pos = np.arange(S)
    causal = pos[:, None] >= pos[None, :]
    structured = bool((lens >= 1).all()) and all(
        np.array_equal(m[b], causal & (pos[None, :] < lens[b])) for b in range(B))
    if not (structured and B == 8 and x.shape == (8, S, D)):
        return _numpy_fallback(x, attn_mask, W_qkv, W_proj, b_proj)
    maxlen = int(lens.max())
    if maxlen not in _RUNNERS:
        nc = build_nc(num_cores=8, maxlen=maxlen)
        _RUNNERS[maxlen] = _build_runner(nc, 8)
    in_maps = [host_prep(x[b], int(lens[b]), W_qkv, W_proj, b_proj)
               for b in range(B)]
    results = _RUNNERS[maxlen](in_maps)
    return np.stack([results[c]["y"] for c in range(8)])
